# revision 40
# baseline (speedup 1.0000x reference)
"""BERT encoder (12 layers, B=8 T=512 D=768 H=12) on 8 Trainium2 NeuronCores.

Strategy: pure data parallelism — core b runs the full 12-layer stack for
batch element b. No collectives. All five per-layer GEMMs run on the tensor
engine in float32r (full-rate fp32); softmax uses ACT Exp with fused
row-sum accumulation; the softmax normalization is applied to P on DVE;
P^T comes from PE transpose-mode; layernorm runs fused on DVE/ACT.

Host-side folds (exact, negligible FLOPs):
  - attention scale 1/sqrt(dh) folded into Wq and bq
  - V bias folded through Wo1: b1 = bv @ Wo1 + bo1 (rows of softmax sum to 1)
  - weights pre-reshaped to the SBUF lhsT chunk layout
Zero biases / zero mask / identity LN affine (which is what
reference.setup_inputs() produces) skip their device ops entirely, but the
general paths are implemented and selected when inputs are nonzero.
"""

import numpy as np

L, B, T, D, H, DH = 12, 8, 512, 768, 12, 64
PD = 128
NKC = D // PD  # 6 contraction chunks
NTC = T // PD  # 4 token chunks
NG = 2         # N-groups per 768-wide output (384 each)
GW = D // NG   # 384
EPS = 1e-12
SCALE = 1.0 / np.sqrt(np.float32(DH))


def _split_excess_waits(nc, mybir, bass_rust, max_waits=1):
    """walrus codegen rejects instructions carrying more than a couple of
    sync waits; hoist excess waits onto same-engine NoOps placed before."""
    n = 0
    for f in nc.m.functions:
        for bb in f.blocks:
            new_insts = []
            changed = False
            for inst in bb.instructions:
                si = inst.sync_info
                if si is not None and len(si.on_wait) > max_waits:
                    waits = list(si.on_wait)
                    excess = waits[: len(waits) - max_waits]
                    for i in range(0, len(excess), max_waits):
                        chunk = excess[i : i + max_waits]
                        n += 1
                        nop = mybir.InstNoOp(
                            name=f"I-waitsplit-{n}", ins=[], outs=[]
                        )
                        nop.engine = inst.engine
                        nop.sync_info = bass_rust.SyncInfo(
                            on_wait=chunk, on_update=[]
                        )
                        new_insts.append(nop)
                        changed = True
                    si.on_wait = waits[len(waits) - max_waits :]
                new_insts.append(inst)
            if changed:
                bb.instructions[:] = new_insts
    return n


def build_nc(flags, split_waits=True):
    """Build the per-core Bass module. flags: dict of general-path toggles."""
    import concourse.bass as bass
    import concourse.tile as tile
    from concourse import mybir

    F32 = mybir.dt.float32
    F32R = mybir.dt.float32r
    BF16 = mybir.dt.bfloat16
    AF = mybir.ActivationFunctionType
    OP = mybir.AluOpType

    use_mask = flags["use_mask"]
    use_bq = flags["use_bq"]
    use_bk = flags["use_bk"]
    use_b1 = flags["use_b1"]
    use_b2 = flags["use_b2"]
    use_ln1 = flags["use_ln1"]
    use_ln2 = flags["use_ln2"]

    nc = bass.Bass("TRN2", target_bir_lowering=False, debug=False)

    qs_d = nc.dram_tensor("qs", [T, D], F32R, kind="ExternalInput")
    hs_d = nc.dram_tensor("hs", [T, D], F32R, kind="ExternalInput")
    w_d = {
        name: nc.dram_tensor(name, [L, PD, NKC * D], F32R, kind="ExternalInput")
        for name in ("wq", "wk", "wv", "wo1", "wo2")
    }
    iden_d = nc.dram_tensor("iden", [PD, PD], F32R, kind="ExternalInput")
    bq_d = nc.dram_tensor("bq", [PD, L * NKC], F32, kind="ExternalInput") if use_bq else None
    bk_d = nc.dram_tensor("bk", [PD, L * NKC], F32, kind="ExternalInput") if use_bk else None
    mask_d = nc.dram_tensor("mask", [PD, NTC], F32, kind="ExternalInput") if use_mask else None
    epair_d = nc.dram_tensor("epair", [H // 2, 3 * PD], BF16, kind="ExternalInput")
    vones_d = nc.dram_tensor("vones", [PD, H], F32R, kind="ExternalInput")
    b1_d = nc.dram_tensor("b1bc", [L, PD, D], F32, kind="ExternalInput") if use_b1 else None
    b2_d = nc.dram_tensor("b2bc", [L, PD, D], F32, kind="ExternalInput") if use_b2 else None
    ln1w_d = nc.dram_tensor("ln1wbc", [L, PD, D], F32, kind="ExternalInput") if use_ln1 else None
    ln1b_d = nc.dram_tensor("ln1bbc", [L, PD, D], F32, kind="ExternalInput") if use_ln1 else None
    ln2w_d = nc.dram_tensor("ln2wbc", [L, PD, D], F32, kind="ExternalInput") if use_ln2 else None
    ln2b_d = nc.dram_tensor("ln2bbc", [L, PD, D], F32, kind="ExternalInput") if use_ln2 else None
    out_d = nc.dram_tensor("out", [T, D], F32R, kind="ExternalOutput")

    evac_ctr = [0]

    with tile.TileContext(nc) as tc:
        import contextlib

        with contextlib.ExitStack() as ctx:
            p_w = ctx.enter_context(tc.tile_pool(name="w", bufs=3))
            p_qt = ctx.enter_context(tc.tile_pool(name="qt", bufs=6))
            p_hid = ctx.enter_context(tc.tile_pool(name="hid", bufs=8))
            p_ht = ctx.enter_context(tc.tile_pool(name="ht", bufs=6))
            p_act = ctx.enter_context(tc.tile_pool(name="act", bufs=12))
            p_ctx = ctx.enter_context(tc.tile_pool(name="ctxp", bufs=7))
            p_v = ctx.enter_context(tc.tile_pool(name="v", bufs=4))
            p_pt = ctx.enter_context(tc.tile_pool(name="pt", bufs=8))
            p_r = ctx.enter_context(tc.tile_pool(name="r", bufs=2))
            p_z = ctx.enter_context(tc.tile_pool(name="z", bufs=2))
            p_sm = ctx.enter_context(tc.tile_pool(name="sm", bufs=2))
            p_c1 = ctx.enter_context(tc.tile_pool(name="c1", bufs=1))
            p_bc = ctx.enter_context(tc.tile_pool(name="bc", bufs=2))
            ps_a = ctx.enter_context(tc.tile_pool(name="psA", bufs=3, space="PSUM"))
            ps_b = ctx.enter_context(tc.tile_pool(name="psB", bufs=2, space="PSUM"))
            ps_c = ctx.enter_context(tc.tile_pool(name="psC", bufs=3, space="PSUM"))

            def evac(dst_ap, src_ap, eng="dve"):
                """PSUM -> SBUF copy. DVE in exp-heavy regions (pair loop),
                ACT for the transpose evacs that run while ACT is idle."""
                if eng == "act":
                    nc.scalar.copy(dst_ap, src_ap)
                else:
                    nc.vector.tensor_copy(dst_ap, src_ap)

            # ---- one-time constants / inputs ----
            iden = p_c1.tile([PD, PD], F32R, tag="iden")
            nc.sync.dma_start(iden[:], iden_d.ap())
            if use_bq:
                bq_t = p_c1.tile([PD, L * NKC], F32, tag="bq")
                nc.sync.dma_start(bq_t[:], bq_d.ap())
            if use_bk:
                bk_t = p_c1.tile([PD, L * NKC], F32, tag="bk")
                nc.sync.dma_start(bk_t[:], bk_d.ap())
            if use_mask:
                mask_t = p_c1.tile([PD, NTC], F32, tag="mask")
                nc.sync.dma_start(mask_t[:], mask_d.ap())
            epair_t = p_c1.tile([H // 2, 3 * PD], BF16, tag="epair")
            nc.sync.dma_start(epair_t[:], epair_d.ap())
            vones_t = p_c1.tile([PD, H], F32R, tag="vones")
            nc.sync.dma_start(vones_t[:], vones_d.ap())

            qs_n = []
            for tc_i in range(NTC):
                t = p_hid.tile([PD, D], F32R, tag="hid")
                nc.sync.dma_start(t[:], qs_d.ap()[tc_i * PD : (tc_i + 1) * PD, :])
                qs_n.append(t)
            h_tiles = []
            for tc_i in range(NTC):
                t = p_hid.tile([PD, D], F32R, tag="hid")
                nc.sync.dma_start(t[:], hs_d.ap()[tc_i * PD : (tc_i + 1) * PD, :])
                h_tiles.append(t)

            def transpose_norm_to_T(src_tiles, pool, tag):
                """[T, D] (4x[128,768] f32r) -> [D, T] (6x[128,512] f32r)."""
                out = []
                for kc in range(NKC):
                    pt = ps_b.tile([PD, T], F32R, tag="pb")
                    for tc_i in range(NTC):
                        nc.tensor.transpose(
                            pt[:, tc_i * PD : (tc_i + 1) * PD],
                            src_tiles[tc_i][:, kc * PD : (kc + 1) * PD],
                            iden[:],
                        )
                    dst = pool.tile([PD, T], F32R, tag=tag)
                    evac(dst[:], pt[:], eng="act")
                    out.append(dst)
                return out

            qT = transpose_norm_to_T(qs_n, p_qt, "qt")

            # Q^T, K^T: [D, T], d_out on partitions, bf16 (feeds the
            # score matmuls whose 64-row stationary tiles only hit the
            # fast PE path for 16-bit operands)
            def proj_chain(w_tile, rhs_tiles, bias_t, use_bias, l, mc):
                pp = ps_a.tile([PD, T], F32, tag="pa")
                for kc in range(NKC):
                    nc.tensor.matmul(
                        pp[:],
                        w_tile[:, kc * D + mc * PD : kc * D + (mc + 1) * PD],
                        rhs_tiles[kc][:],
                        start=(kc == 0),
                        stop=(kc == NKC - 1),
                    )
                dst = p_act.tile([PD, T], BF16, tag="qk")
                if use_bias:
                    nc.scalar.activation(
                        dst[:], pp[:], AF.Identity,
                        bias=bias_t[:, l * NKC + mc : l * NKC + mc + 1],
                        scale=1.0,
                    )
                else:
                    evac(dst[:], pp[:])
                return dst

            def proj_T(w_tile, rhs_tiles, bias_t, use_bias, l):
                return [
                    proj_chain(w_tile, rhs_tiles, bias_t, use_bias, l, mc)
                    for mc in range(NKC)
                ]

            # Q projection for layer 0 up front; later layers' Q proj is
            # emitted during layer l-1's softmax-denominator window so the
            # PE never idles there (query_states is layer-invariant).
            wq_t = p_w.tile([PD, NKC * D], F32R, tag="w")
            nc.sync.dma_start(wq_t[:], w_d["wq"].ap()[0])
            QT = proj_T(wq_t, qT, bq_t if use_bq else None, use_bq, 0)

            # ---- layers ----
            for l in range(L):
                wk_t = p_w.tile([PD, NKC * D], F32R, tag="w")
                nc.sync.dma_start(wk_t[:], w_d["wk"].ap()[l])
                wv_t = p_w.tile([PD, NKC * D], F32R, tag="w")
                nc.sync.dma_start(wv_t[:], w_d["wv"].ap()[l])

                hT = transpose_norm_to_T(h_tiles, p_ht, "ht")

                KT = proj_T(wk_t, hT, bk_t if use_bk else None, use_bk, l)

                # V: augmented normal layout [k, 12*65]; head h at cols
                # 65h..65h+63, ones at col 65h+64 (emits the softmax
                # denominator as row 64 of the PV product).
                V = []
                for tc_i in range(NTC):
                    vt = p_v.tile([PD, H * 65], BF16, tag="v")
                    for ng in range(NG):
                        pp = ps_b.tile([PD, GW], F32, tag="pb")
                        for kc in range(NKC):
                            nc.tensor.matmul(
                                pp[:],
                                hT[kc][:, tc_i * PD : (tc_i + 1) * PD],
                                wv_t[:, kc * D + ng * GW : kc * D + (ng + 1) * GW],
                                start=(kc == 0),
                                stop=(kc == NKC - 1),
                            )
                        dst = vt[:, ng * 390 : (ng + 1) * 390].rearrange(
                            "p (h c) -> p h c", c=65
                        )[:, :, 0:64]
                        src_ = pp[:].rearrange("p (h c) -> p h c", c=64)
                        evac(dst, src_)
                    ones_dst = vt[:].rearrange("p (h c) -> p h c", c=65)[:, :, 64:65]
                    nc.vector.tensor_copy(
                        ones_dst, vones_t[:].rearrange("p (h o) -> p h o", o=1)
                    )
                    V.append(vt)

                wo1_t = p_w.tile([PD, NKC * D], F32R, tag="w")
                nc.sync.dma_start(wo1_t[:], w_d["wo1"].ap()[l])
                wo2_t = p_w.tile([PD, NKC * D], F32R, tag="w")
                nc.sync.dma_start(wo2_t[:], w_d["wo2"].ap()[l])

                ctxT = [
                    p_ctx.tile([PD, T], F32R, tag="ctx", name=f"ctx{i}")
                    for i in range(NKC)
                ]
                craw = [
                    p_ctx.tile([PD, T], F32R, tag="ctxr", bufs=6,
                               name=f"cr{i}")
                    for i in range(NKC)
                ]

                # softmax denominators get stacked on partitions 0-5 of two
                # half-layer tiles (tiny SBUF->SBUF DMAs; engine writes
                # can't start at arbitrary partitions but DMA can) -> one
                # DVE reciprocal per 6 heads (InstReciprocal costs
                # ~4cyc/free-elem regardless of partition count). Two
                # halves so the first recip + normalize chain overlaps the
                # second half of the pair loop.
                dhalf = [
                    p_sm.tile([H // 2, T], F32, tag=f"dall{i}", bufs=2,
                              name=f"dall{i}_{l}")
                    for i in range(2)
                ]
                rhalf = [None, None]

                # next layer's Q-projection chains are interleaved below:
                # the exp pipeline on ACT paces the pair loop, and without
                # filler the PE idles and its clock gate drops to half
                # rate (HAM K=4)
                if l + 1 < L:
                    wq_nt = p_w.tile([PD, NKC * D], F32R, tag="w")
                    nc.sync.dma_start(wq_nt[:], w_d["wq"].ap()[l + 1])
                else:
                    wq_nt = None
                QT_next = []

                def qtn_chain():
                    if wq_nt is not None and len(QT_next) < NKC:
                        QT_next.append(
                            proj_chain(wq_nt, qT, bq_t if use_bq else None,
                                       use_bq, l + 1, len(QT_next))
                        )

                def consume(p):
                    """Broadcast 1/den for pair p and normalize its ctx.
                    The multiplies run on the (otherwise idle) GPSIMD, so
                    pr bounces through SBUF (GPSIMD has no PSUM port)."""
                    half, row0 = divmod(2 * p, H // 2)
                    pr = ps_b.tile([PD, T], F32, tag="pb", name=f"pr{p}")
                    nc.tensor.matmul(
                        pr[:],
                        epair_t[:, (row0 // 2) * PD : (row0 // 2 + 1) * PD],
                        rhalf[half][:],
                        start=True,
                        stop=True,
                    )
                    prsb = p_r.tile([PD, T], BF16, tag="rsb", name=f"prs{p}")
                    nc.vector.tensor_copy(prsb[:], pr[:])
                    for sub in range(2):
                        off = 64 * sub
                        nc.gpsimd.tensor_tensor(
                            ctxT[p][off : off + 64, :],
                            craw[p][off : off + 64, :],
                            prsb[off : off + 64, :],
                            op=OP.mult,
                        )

                for pair in range(H // 2):
                    h0, h1 = pair * 2, pair * 2 + 1
                    qtile = QT[pair]
                    ktile = KT[pair]
                    # both heads' score matmuls first, so the ACT exp
                    # pipeline runs ahead of the PV accumulation chain
                    pts = {}
                    for sub in range(2):
                        hh = pair * 2 + sub
                        off = 64 * sub
                        for kb in range(NTC):
                            sp = ps_a.tile([PD, T], F32, tag="pa", name=f"sp{hh}_{kb}")
                            nc.tensor.matmul(
                                sp[:],
                                ktile[off : off + 64, kb * PD : (kb + 1) * PD],
                                qtile[off : off + 64, :],
                                start=True,
                                stop=True,
                            )
                            pt = p_pt.tile([PD, T], BF16, tag="pts",
                                           name=f"pt{hh}_{kb}")
                            if use_mask:
                                nc.scalar.activation(
                                    pt[:], sp[:], AF.Exp,
                                    bias=mask_t[:, kb : kb + 1], scale=1.0,
                                )
                            else:
                                nc.scalar.activation(
                                    pt[:], sp[:], AF.Exp, bias=0.0, scale=1.0,
                                )
                            pts[(sub, kb)] = pt
                    # PV accumulation; V row 64 of each head is ones, so
                    # cp row 64 is the softmax denominator for that head.
                    for sub in range(2):
                        hh = pair * 2 + sub
                        off = 64 * sub
                        cp = ps_c.tile([65, T], F32, tag="ctxp", name=f"cp{hh}")
                        for kb in range(NTC):
                            nc.tensor.matmul(
                                cp[:],
                                V[kb][:, 65 * hh : 65 * hh + 65],
                                pts[(sub, kb)][:],
                                start=(kb == 0),
                                stop=(kb == NTC - 1),
                            )
                        den = p_sm.tile([1, T], F32, tag="den", bufs=4,
                                        name=f"den{hh}")
                        nc.vector.tensor_copy(den[:], cp[64:65, :])
                        nc.sync.dma_start(
                            dhalf[hh // 6][hh % 6 : hh % 6 + 1, :], den[:]
                        )
                        # unnormalized ctx -> SBUF (frees the PSUM bank;
                        # normalization happens after the batched recip)
                        evac(craw[pair][off : off + 64, :], cp[0:64, :])

                    if pair == 2 or pair == 5:
                        half = pair // 3
                        rhalf[half] = p_sm.tile(
                            [H // 2, T], BF16, tag=f"rall{half}", bufs=2,
                            name=f"rall{half}_{l}",
                        )
                        with nc.allow_low_precision("softmax denom bf16"):
                            nc.vector.reciprocal(
                                rhalf[half][:], dhalf[half][:]
                            )
                    if pair >= 3:
                        qtn_chain()
                        consume(pair - 3)

                qtn_chain()
                qtn_chain()
                consume(3)
                consume(4)
                consume(5)

                # ---- output block: z = x @ W + residual, then LN ----
                def out_block(lhsT_tiles, w_tile, res_tiles, badd_d, use_badd,
                              lnw_d_, lnb_d_, use_ln, out_tag, is_last):
                    outs = []
                    if use_badd:
                        badd_t = p_bc.tile([PD, D], F32, tag="badd")
                        nc.sync.dma_start(badd_t[:], badd_d.ap()[l])
                    if use_ln:
                        lnw_t = p_bc.tile([PD, D], F32, tag="lnw")
                        nc.sync.dma_start(lnw_t[:], lnw_d_.ap()[l])
                        lnb_t = p_bc.tile([PD, D], F32, tag="lnb")
                        nc.sync.dma_start(lnb_t[:], lnb_d_.ap()[l])
                    for tc_i in range(NTC):
                        z = p_z.tile([PD, D], F32, tag="z")
                        s01 = p_sm.tile([PD, NG], F32, tag="s01")
                        for ng in range(NG):
                            pp = ps_b.tile([PD, GW], F32, tag="pb")
                            for kc in range(NKC):
                                nc.tensor.matmul(
                                    pp[:],
                                    lhsT_tiles[kc][:, tc_i * PD : (tc_i + 1) * PD],
                                    w_tile[:, kc * D + ng * GW : kc * D + (ng + 1) * GW],
                                    start=(kc == 0),
                                    stop=(kc == NKC - 1),
                                )
                            sl = slice(ng * GW, (ng + 1) * GW)
                            if use_badd:
                                nc.vector.scalar_tensor_tensor(
                                    z[:, sl], pp[:], 1.0, res_tiles[tc_i][:, sl],
                                    op0=OP.mult, op1=OP.add,
                                )
                                nc.vector.scalar_tensor_tensor(
                                    z[:, sl], z[:, sl], 1.0, badd_t[:, sl],
                                    op0=OP.mult, op1=OP.add,
                                    accum_out=s01[:, ng : ng + 1],
                                )
                            else:
                                nc.vector.scalar_tensor_tensor(
                                    z[:, sl], pp[:], 1.0, res_tiles[tc_i][:, sl],
                                    op0=OP.mult, op1=OP.add,
                                    accum_out=s01[:, ng : ng + 1],
                                )
                        # layernorm over the full 768-wide row
                        ssum = p_sm.tile([PD, 1], F32, tag="ssum")
                        nc.vector.tensor_tensor(
                            ssum[:], s01[:, 0:1], s01[:, 1:2], op=OP.add
                        )
                        uneg = p_sm.tile([PD, 1], F32, tag="uneg")
                        nc.vector.tensor_scalar_mul(uneg[:], ssum[:], -1.0 / D)
                        sq = p_z.tile([PD, D], F32, tag="sq")
                        ssq = p_sm.tile([PD, 1], F32, tag="ssq")
                        nc.scalar.activation(
                            sq[:], z[:], AF.Square, bias=uneg[:], scale=1.0,
                            accum_out=ssq[:],
                        )
                        # rstd = exp(-0.5*ln(ssq/D)): Ln+Exp share one ACT
                        # table set with the attention Exp (Sqrt does not,
                        # and each set switch costs ~2.7us); /D folds into
                        # the Ln scale (reference eps=1e-12 is below fp32
                        # resolution for var ~ O(1), so it is dropped)
                        lnv = p_sm.tile([PD, 1], F32, tag="stdev")
                        nc.scalar.activation(
                            lnv[:], ssq[:], AF.Ln, bias=0.0, scale=1.0 / D
                        )
                        rstd = p_sm.tile([PD, 1], F32, tag="rstd")
                        nc.scalar.activation(
                            rstd[:], lnv[:], AF.Exp, bias=0.0, scale=-0.5
                        )
                        o = p_hid.tile([PD, D], F32R, tag=out_tag)
                        if use_ln:
                            on = p_z.tile([PD, D], F32, tag="sq")
                            nc.vector.tensor_scalar(
                                on[:], z[:], uneg[:], rstd[:], op0=OP.add, op1=OP.mult
                            )
                            nc.vector.tensor_tensor(
                                on[:], on[:], lnw_t[:], op=OP.mult
                            )
                            nc.vector.tensor_tensor(
                                o[:], on[:], lnb_t[:], op=OP.add
                            )
                        else:
                            nc.vector.tensor_scalar(
                                o[:], z[:], uneg[:], rstd[:], op0=OP.add, op1=OP.mult
                            )
                        if is_last:
                            nc.sync.dma_start(
                                out_d.ap()[tc_i * PD : (tc_i + 1) * PD, :], o[:]
                            )
                        outs.append(o)
                    return outs

                a_tiles = out_block(
                    ctxT, wo1_t, h_tiles, b1_d, use_b1,
                    ln1w_d, ln1b_d, use_ln1, "hid", False,
                )
                aT = transpose_norm_to_T(a_tiles, p_ht, "ht")
                h_tiles = out_block(
                    aT, wo2_t, a_tiles, b2_d, use_b2,
                    ln2w_d, ln2b_d, use_ln2, "hid", l == L - 1,
                )
                # last Q-proj chain lands after out2: it covers the PE
                # window where the next layer's hT transposes wait on the
                # final LN tiles
                qtn_chain()
                if l + 1 < L:
                    assert len(QT_next) == NKC
                    QT = QT_next

    if split_waits:
        import bass_rust

        _split_excess_waits(nc, mybir, bass_rust)
    return nc


def prep_inputs(inputs):
    """Host-side folds. Returns (flags, per-core-invariant map, per-core list)."""
    g = {k: np.asarray(v, dtype=np.float32) for k, v in inputs.items()}

    wq_s = g["Wq"] * SCALE
    bq_s = g["bq"] * SCALE
    b1 = np.einsum("ld,ldo->lo", g["bv"], g["Wo1"]) + g["bo1"]
    b2 = g["bo2"]

    flags = {
        "use_mask": bool(np.any(g["attention_mask"])),
        "use_bq": bool(np.any(bq_s)),
        "use_bk": bool(np.any(g["bk"])),
        "use_b1": bool(np.any(b1)),
        "use_b2": bool(np.any(b2)),
        "use_ln1": bool(np.any(g["ln1_w"] != 1.0) or np.any(g["ln1_b"])),
        "use_ln2": bool(np.any(g["ln2_w"] != 1.0) or np.any(g["ln2_b"])),
    }

    def wfmt(w):
        return np.ascontiguousarray(
            w.reshape(L, NKC, PD, D).transpose(0, 2, 1, 3).reshape(L, PD, NKC * D)
        )

    def bfmt(b):
        return np.ascontiguousarray(
            b.reshape(L, NKC, PD).transpose(2, 0, 1).reshape(PD, L * NKC)
        )

    shared = {
        "wq": wfmt(wq_s),
        "wk": wfmt(g["Wk"]),
        "wv": wfmt(g["Wv"]),
        "wo1": wfmt(g["Wo1"]),
        "wo2": wfmt(g["Wo2"]),
        "iden": np.eye(PD, dtype=np.float32),
    }
    if flags["use_bq"]:
        shared["bq"] = bfmt(bq_s)
    if flags["use_bk"]:
        shared["bk"] = bfmt(g["bk"])
    import ml_dtypes

    epair = np.zeros((H // 2, 3 * PD), dtype=ml_dtypes.bfloat16)
    for r in range(3):
        epair[2 * r, r * PD : r * PD + 64] = 1.0
        epair[2 * r + 1, r * PD + 64 : (r + 1) * PD] = 1.0
    shared["epair"] = epair
    shared["vones"] = np.ones((PD, H), dtype=np.float32)
    if flags["use_b1"]:
        shared["b1bc"] = np.ascontiguousarray(
            np.broadcast_to(b1[:, None, :], (L, PD, D))
        )
    if flags["use_b2"]:
        shared["b2bc"] = np.ascontiguousarray(
            np.broadcast_to(b2[:, None, :], (L, PD, D))
        )
    if flags["use_ln1"]:
        shared["ln1wbc"] = np.ascontiguousarray(
            np.broadcast_to(g["ln1_w"][:, None, :], (L, PD, D))
        )
        shared["ln1bbc"] = np.ascontiguousarray(
            np.broadcast_to(g["ln1_b"][:, None, :], (L, PD, D))
        )
    if flags["use_ln2"]:
        shared["ln2wbc"] = np.ascontiguousarray(
            np.broadcast_to(g["ln2_w"][:, None, :], (L, PD, D))
        )
        shared["ln2bbc"] = np.ascontiguousarray(
            np.broadcast_to(g["ln2_b"][:, None, :], (L, PD, D))
        )

    per_core = []
    for b in range(B):
        m = dict(shared)
        m["qs"] = np.ascontiguousarray(g["query_states"][b])
        m["hs"] = np.ascontiguousarray(g["hidden_states"][b])
        if flags["use_mask"]:
            m["mask"] = np.ascontiguousarray(
                g["attention_mask"][b].reshape(NTC, PD).T
            )
        per_core.append(m)
    return flags, per_core


TRACE = False
LAST_EXEC_NS = None
LAST_RESULTS = None


def kernel(**inputs):
    global LAST_EXEC_NS, LAST_RESULTS
    from concourse.bass_utils import run_bass_kernel_spmd

    flags, per_core = prep_inputs(inputs)
    nc = build_nc(flags)
    kw = {}
    if TRACE:
        kw = dict(trace=True, tmpdir="/root/problem/trace_out")
        import os

        os.makedirs("/root/problem/trace_out", exist_ok=True)
    res = run_bass_kernel_spmd(nc, per_core, core_ids=list(range(B)), **kw)
    LAST_EXEC_NS = res.exec_time_ns
    LAST_RESULTS = res
    out = np.stack([np.asarray(res.results[b]["out"]) for b in range(B)], axis=0)
    return out.astype(np.float32)



# revision 47
# speedup vs baseline: 1.0451x; 1.0451x over previous
"""BERT encoder (12 layers, B=8 T=512 D=768 H=12) on 8 Trainium2 NeuronCores.

Strategy: pure data parallelism — core b runs the full 12-layer stack for
batch element b. No collectives. All five per-layer GEMMs run on the tensor
engine in float32r (full-rate fp32); softmax uses ACT Exp with fused
row-sum accumulation; the softmax normalization is applied to P on DVE;
P^T comes from PE transpose-mode; layernorm runs fused on DVE/ACT.

Host-side folds (exact, negligible FLOPs):
  - attention scale 1/sqrt(dh) folded into Wq and bq
  - V bias folded through Wo1: b1 = bv @ Wo1 + bo1 (rows of softmax sum to 1)
  - weights pre-reshaped to the SBUF lhsT chunk layout
Zero biases / zero mask / identity LN affine (which is what
reference.setup_inputs() produces) skip their device ops entirely, but the
general paths are implemented and selected when inputs are nonzero.
"""

import numpy as np

L, B, T, D, H, DH = 12, 8, 512, 768, 12, 64
PD = 128
NKC = D // PD  # 6 contraction chunks
NTC = T // PD  # 4 token chunks
NG = 2         # N-groups per 768-wide output (384 each)
GW = D // NG   # 384
EPS = 1e-12
SCALE = 1.0 / np.sqrt(np.float32(DH))


def _split_excess_waits(nc, mybir, bass_rust, max_waits=1):
    """walrus codegen rejects instructions carrying more than a couple of
    sync waits; hoist excess waits onto same-engine NoOps placed before."""
    n = 0
    for f in nc.m.functions:
        for bb in f.blocks:
            new_insts = []
            changed = False
            for inst in bb.instructions:
                si = inst.sync_info
                if si is not None and len(si.on_wait) > max_waits:
                    waits = list(si.on_wait)
                    excess = waits[: len(waits) - max_waits]
                    for i in range(0, len(excess), max_waits):
                        chunk = excess[i : i + max_waits]
                        n += 1
                        nop = mybir.InstNoOp(
                            name=f"I-waitsplit-{n}", ins=[], outs=[]
                        )
                        nop.engine = inst.engine
                        nop.sync_info = bass_rust.SyncInfo(
                            on_wait=chunk, on_update=[]
                        )
                        new_insts.append(nop)
                        changed = True
                    si.on_wait = waits[len(waits) - max_waits :]
                new_insts.append(inst)
            if changed:
                bb.instructions[:] = new_insts
    return n


def build_nc(flags, split_waits=True):
    """Build the per-core Bass module. flags: dict of general-path toggles."""
    import concourse.bass as bass
    import concourse.tile as tile
    from concourse import mybir

    F32 = mybir.dt.float32
    F32R = mybir.dt.float32r
    BF16 = mybir.dt.bfloat16
    AF = mybir.ActivationFunctionType
    OP = mybir.AluOpType

    use_mask = flags["use_mask"]
    use_bq = flags["use_bq"]
    use_bk = flags["use_bk"]
    use_b1 = flags["use_b1"]
    use_b2 = flags["use_b2"]
    use_ln1 = flags["use_ln1"]
    use_ln2 = flags["use_ln2"]

    nc = bass.Bass("TRN2", target_bir_lowering=False, debug=False)

    qs_d = nc.dram_tensor("qs", [T, D], F32R, kind="ExternalInput")
    hs_d = nc.dram_tensor("hs", [T, D], F32R, kind="ExternalInput")
    w_d = {
        name: nc.dram_tensor(name, [L, PD, NKC * D], BF16, kind="ExternalInput")
        for name in ("wq", "wk", "wv", "wo1", "wo2")
    }
    iden_d = nc.dram_tensor("iden", [PD, PD], F32R, kind="ExternalInput")
    bq_d = nc.dram_tensor("bq", [PD, L * NKC], F32, kind="ExternalInput") if use_bq else None
    bk_d = nc.dram_tensor("bk", [PD, L * NKC], F32, kind="ExternalInput") if use_bk else None
    mask_d = nc.dram_tensor("mask", [PD, NTC], F32, kind="ExternalInput") if use_mask else None
    epair_d = nc.dram_tensor("epair", [H // 2, 3 * PD], BF16, kind="ExternalInput")
    vones_d = nc.dram_tensor("vones", [PD, H], F32R, kind="ExternalInput")
    b1_d = nc.dram_tensor("b1bc", [L, PD, D], F32, kind="ExternalInput") if use_b1 else None
    b2_d = nc.dram_tensor("b2bc", [L, PD, D], F32, kind="ExternalInput") if use_b2 else None
    ln1w_d = nc.dram_tensor("ln1wbc", [L, PD, D], F32, kind="ExternalInput") if use_ln1 else None
    ln1b_d = nc.dram_tensor("ln1bbc", [L, PD, D], F32, kind="ExternalInput") if use_ln1 else None
    ln2w_d = nc.dram_tensor("ln2wbc", [L, PD, D], F32, kind="ExternalInput") if use_ln2 else None
    ln2b_d = nc.dram_tensor("ln2bbc", [L, PD, D], F32, kind="ExternalInput") if use_ln2 else None
    out_d = nc.dram_tensor("out", [T, D], F32R, kind="ExternalOutput")

    evac_ctr = [0]

    with tile.TileContext(nc) as tc:
        import contextlib

        with contextlib.ExitStack() as ctx:
            p_w = ctx.enter_context(tc.tile_pool(name="w", bufs=3))
            p_qt = ctx.enter_context(tc.tile_pool(name="qt", bufs=6))
            p_hid = ctx.enter_context(tc.tile_pool(name="hid", bufs=8))
            p_ht = ctx.enter_context(tc.tile_pool(name="ht", bufs=6))
            p_act = ctx.enter_context(tc.tile_pool(name="act", bufs=12))
            p_ctx = ctx.enter_context(tc.tile_pool(name="ctxp", bufs=7))
            p_v = ctx.enter_context(tc.tile_pool(name="v", bufs=4))
            p_pt = ctx.enter_context(tc.tile_pool(name="pt", bufs=8))
            p_r = ctx.enter_context(tc.tile_pool(name="r", bufs=2))
            p_z = ctx.enter_context(tc.tile_pool(name="z", bufs=2))
            p_sm = ctx.enter_context(tc.tile_pool(name="sm", bufs=2))
            p_c1 = ctx.enter_context(tc.tile_pool(name="c1", bufs=1))
            p_bc = ctx.enter_context(tc.tile_pool(name="bc", bufs=2))
            ps_a = ctx.enter_context(tc.tile_pool(name="psA", bufs=3, space="PSUM"))
            ps_b = ctx.enter_context(tc.tile_pool(name="psB", bufs=2, space="PSUM"))
            ps_c = ctx.enter_context(tc.tile_pool(name="psC", bufs=3, space="PSUM"))

            def evac(dst_ap, src_ap, eng="dve"):
                """PSUM -> SBUF copy. DVE in exp-heavy regions (pair loop),
                ACT for the transpose evacs that run while ACT is idle."""
                if eng == "act":
                    nc.scalar.copy(dst_ap, src_ap)
                else:
                    nc.vector.tensor_copy(dst_ap, src_ap)

            # ---- one-time constants / inputs ----
            iden = p_c1.tile([PD, PD], F32R, tag="iden")
            nc.sync.dma_start(iden[:], iden_d.ap())
            if use_bq:
                bq_t = p_c1.tile([PD, L * NKC], F32, tag="bq")
                nc.sync.dma_start(bq_t[:], bq_d.ap())
            if use_bk:
                bk_t = p_c1.tile([PD, L * NKC], F32, tag="bk")
                nc.sync.dma_start(bk_t[:], bk_d.ap())
            if use_mask:
                mask_t = p_c1.tile([PD, NTC], F32, tag="mask")
                nc.sync.dma_start(mask_t[:], mask_d.ap())
            epair_t = p_c1.tile([H // 2, 3 * PD], BF16, tag="epair")
            nc.sync.dma_start(epair_t[:], epair_d.ap())
            vones_t = p_c1.tile([PD, H], F32R, tag="vones")
            nc.sync.dma_start(vones_t[:], vones_d.ap())

            qs_n = []
            for tc_i in range(NTC):
                t = p_hid.tile([PD, D], F32R, tag="hid")
                nc.sync.dma_start(t[:], qs_d.ap()[tc_i * PD : (tc_i + 1) * PD, :])
                qs_n.append(t)
            h_tiles = []
            for tc_i in range(NTC):
                t = p_hid.tile([PD, D], F32R, tag="hid")
                nc.sync.dma_start(t[:], hs_d.ap()[tc_i * PD : (tc_i + 1) * PD, :])
                h_tiles.append(t)

            def transpose_norm_to_T(src_tiles, pool, tag):
                """[T, D] (4x[128,768] bf16) -> [D, T] (6x[128,512] bf16)."""
                out = []
                for kc in range(NKC):
                    pt = ps_b.tile([PD, T], F32R, tag="pb")
                    for tc_i in range(NTC):
                        nc.tensor.transpose(
                            pt[:, tc_i * PD : (tc_i + 1) * PD],
                            src_tiles[tc_i][:, kc * PD : (kc + 1) * PD],
                            iden[:],
                        )
                    dst = pool.tile([PD, T], BF16, tag=tag)
                    evac(dst[:], pt[:], eng="act")
                    out.append(dst)
                return out

            qT = transpose_norm_to_T(qs_n, p_qt, "qt")

            # Q^T, K^T: [D, T], d_out on partitions, bf16 (feeds the
            # score matmuls whose 64-row stationary tiles only hit the
            # fast PE path for 16-bit operands)
            def proj_chain(w_tile, rhs_tiles, bias_t, use_bias, l, mc):
                pp = ps_a.tile([PD, T], F32, tag="pa")
                for kc in range(NKC):
                    nc.tensor.matmul(
                        pp[:],
                        w_tile[:, kc * D + mc * PD : kc * D + (mc + 1) * PD],
                        rhs_tiles[kc][:],
                        start=(kc == 0),
                        stop=(kc == NKC - 1),
                    )
                dst = p_act.tile([PD, T], BF16, tag="qk")
                if use_bias:
                    nc.scalar.activation(
                        dst[:], pp[:], AF.Identity,
                        bias=bias_t[:, l * NKC + mc : l * NKC + mc + 1],
                        scale=1.0,
                    )
                else:
                    evac(dst[:], pp[:])
                return dst

            def proj_T(w_tile, rhs_tiles, bias_t, use_bias, l):
                return [
                    proj_chain(w_tile, rhs_tiles, bias_t, use_bias, l, mc)
                    for mc in range(NKC)
                ]

            # Q projection for layer 0 up front; later layers' Q proj is
            # emitted during layer l-1's softmax-denominator window so the
            # PE never idles there (query_states is layer-invariant).
            wq_t = p_w.tile([PD, NKC * D], BF16, tag="w")
            nc.sync.dma_start(wq_t[:], w_d["wq"].ap()[0])
            QT = proj_T(wq_t, qT, bq_t if use_bq else None, use_bq, 0)

            # ---- layers ----
            for l in range(L):
                wk_t = p_w.tile([PD, NKC * D], BF16, tag="w")
                nc.sync.dma_start(wk_t[:], w_d["wk"].ap()[l])
                wv_t = p_w.tile([PD, NKC * D], BF16, tag="w")
                nc.sync.dma_start(wv_t[:], w_d["wv"].ap()[l])

                hT = transpose_norm_to_T(h_tiles, p_ht, "ht")

                KT = proj_T(wk_t, hT, bk_t if use_bk else None, use_bk, l)

                # V: augmented normal layout [k, 12*65]; head h at cols
                # 65h..65h+63, ones at col 65h+64 (emits the softmax
                # denominator as row 64 of the PV product).
                V = []
                for tc_i in range(NTC):
                    vt = p_v.tile([PD, H * 65], BF16, tag="v")
                    for ng in range(NG):
                        pp = ps_b.tile([PD, GW], F32, tag="pb")
                        for kc in range(NKC):
                            nc.tensor.matmul(
                                pp[:],
                                hT[kc][:, tc_i * PD : (tc_i + 1) * PD],
                                wv_t[:, kc * D + ng * GW : kc * D + (ng + 1) * GW],
                                start=(kc == 0),
                                stop=(kc == NKC - 1),
                            )
                        dst = vt[:, ng * 390 : (ng + 1) * 390].rearrange(
                            "p (h c) -> p h c", c=65
                        )[:, :, 0:64]
                        src_ = pp[:].rearrange("p (h c) -> p h c", c=64)
                        evac(dst, src_)
                    ones_dst = vt[:].rearrange("p (h c) -> p h c", c=65)[:, :, 64:65]
                    nc.vector.tensor_copy(
                        ones_dst, vones_t[:].rearrange("p (h o) -> p h o", o=1)
                    )
                    V.append(vt)

                wo1_t = p_w.tile([PD, NKC * D], BF16, tag="w")
                nc.sync.dma_start(wo1_t[:], w_d["wo1"].ap()[l])
                wo2_t = p_w.tile([PD, NKC * D], BF16, tag="w")
                nc.sync.dma_start(wo2_t[:], w_d["wo2"].ap()[l])

                ctxT = [
                    p_ctx.tile([PD, T], BF16, tag="ctx", name=f"ctx{i}")
                    for i in range(NKC)
                ]
                craw = [
                    p_ctx.tile([PD, T], BF16, tag="ctxr", bufs=6,
                               name=f"cr{i}")
                    for i in range(NKC)
                ]

                # softmax denominators get stacked on partitions 0-5 of two
                # half-layer tiles (tiny SBUF->SBUF DMAs; engine writes
                # can't start at arbitrary partitions but DMA can) -> one
                # DVE reciprocal per 6 heads (InstReciprocal costs
                # ~4cyc/free-elem regardless of partition count). Two
                # halves so the first recip + normalize chain overlaps the
                # second half of the pair loop.
                dhalf = [
                    p_sm.tile([H // 2, T], F32, tag=f"dall{i}", bufs=2,
                              name=f"dall{i}_{l}")
                    for i in range(2)
                ]
                rhalf = [None, None]

                # next layer's Q-projection chains are interleaved below:
                # the exp pipeline on ACT paces the pair loop, and without
                # filler the PE idles and its clock gate drops to half
                # rate (HAM K=4)
                if l + 1 < L:
                    wq_nt = p_w.tile([PD, NKC * D], BF16, tag="w")
                    nc.sync.dma_start(wq_nt[:], w_d["wq"].ap()[l + 1])
                else:
                    wq_nt = None
                QT_next = []

                def qtn_chain():
                    if wq_nt is not None and len(QT_next) < NKC:
                        QT_next.append(
                            proj_chain(wq_nt, qT, bq_t if use_bq else None,
                                       use_bq, l + 1, len(QT_next))
                        )

                def consume(p):
                    """Broadcast 1/den for pair p and normalize its ctx.
                    The multiplies run on the (otherwise idle) GPSIMD, so
                    pr bounces through SBUF (GPSIMD has no PSUM port)."""
                    half, row0 = divmod(2 * p, H // 2)
                    pr = ps_b.tile([PD, T], F32, tag="pb", name=f"pr{p}")
                    nc.tensor.matmul(
                        pr[:],
                        epair_t[:, (row0 // 2) * PD : (row0 // 2 + 1) * PD],
                        rhalf[half][:],
                        start=True,
                        stop=True,
                    )
                    for sub in range(2):
                        off = 64 * sub
                        nc.vector.tensor_tensor(
                            ctxT[p][off : off + 64, :],
                            craw[p][off : off + 64, :],
                            pr[off : off + 64, :],
                            op=OP.mult,
                        )

                for pair in range(H // 2):
                    h0, h1 = pair * 2, pair * 2 + 1
                    qtile = QT[pair]
                    ktile = KT[pair]
                    # both heads' score matmuls first, so the ACT exp
                    # pipeline runs ahead of the PV accumulation chain
                    pts = {}
                    for sub in range(2):
                        hh = pair * 2 + sub
                        off = 64 * sub
                        for kb in range(NTC):
                            sp = ps_a.tile([PD, T], F32, tag="pa", name=f"sp{hh}_{kb}")
                            nc.tensor.matmul(
                                sp[:],
                                ktile[off : off + 64, kb * PD : (kb + 1) * PD],
                                qtile[off : off + 64, :],
                                start=True,
                                stop=True,
                            )
                            pt = p_pt.tile([PD, T], BF16, tag="pts",
                                           name=f"pt{hh}_{kb}")
                            if use_mask:
                                nc.scalar.activation(
                                    pt[:], sp[:], AF.Exp,
                                    bias=mask_t[:, kb : kb + 1], scale=1.0,
                                )
                            else:
                                nc.scalar.activation(
                                    pt[:], sp[:], AF.Exp, bias=0.0, scale=1.0,
                                )
                            pts[(sub, kb)] = pt
                    # PV accumulation; V row 64 of each head is ones, so
                    # cp row 64 is the softmax denominator for that head.
                    for sub in range(2):
                        hh = pair * 2 + sub
                        off = 64 * sub
                        cp = ps_c.tile([65, T], F32, tag="ctxp", name=f"cp{hh}")
                        for kb in range(NTC):
                            nc.tensor.matmul(
                                cp[:],
                                V[kb][:, 65 * hh : 65 * hh + 65],
                                pts[(sub, kb)][:],
                                start=(kb == 0),
                                stop=(kb == NTC - 1),
                            )
                        den = p_sm.tile([1, T], F32, tag="den", bufs=4,
                                        name=f"den{hh}")
                        nc.vector.tensor_copy(den[:], cp[64:65, :])
                        nc.sync.dma_start(
                            dhalf[hh // 6][hh % 6 : hh % 6 + 1, :], den[:]
                        )
                        # unnormalized ctx -> SBUF (frees the PSUM bank;
                        # normalization happens after the batched recip)
                        evac(craw[pair][off : off + 64, :], cp[0:64, :])

                    if pair == 2 or pair == 5:
                        half = pair // 3
                        rhalf[half] = p_sm.tile(
                            [H // 2, T], BF16, tag=f"rall{half}", bufs=2,
                            name=f"rall{half}_{l}",
                        )
                        with nc.allow_low_precision("softmax denom bf16"):
                            nc.vector.reciprocal(
                                rhalf[half][:], dhalf[half][:]
                            )
                    if pair >= 3:
                        qtn_chain()
                        consume(pair - 3)

                qtn_chain()
                qtn_chain()
                consume(3)
                consume(4)
                consume(5)

                # ---- output block: z = x @ W + residual, then LN ----
                def out_block(lhsT_tiles, w_tile, res_tiles, badd_d, use_badd,
                              lnw_d_, lnb_d_, use_ln, out_tag, is_last):
                    outs = []
                    if use_badd:
                        badd_t = p_bc.tile([PD, D], F32, tag="badd")
                        nc.sync.dma_start(badd_t[:], badd_d.ap()[l])
                    if use_ln:
                        lnw_t = p_bc.tile([PD, D], F32, tag="lnw")
                        nc.sync.dma_start(lnw_t[:], lnw_d_.ap()[l])
                        lnb_t = p_bc.tile([PD, D], F32, tag="lnb")
                        nc.sync.dma_start(lnb_t[:], lnb_d_.ap()[l])
                    for tc_i in range(NTC):
                        z = p_z.tile([PD, D], F32, tag="z")
                        s01 = p_sm.tile([PD, NG], F32, tag="s01")
                        for ng in range(NG):
                            pp = ps_b.tile([PD, GW], F32, tag="pb")
                            for kc in range(NKC):
                                nc.tensor.matmul(
                                    pp[:],
                                    lhsT_tiles[kc][:, tc_i * PD : (tc_i + 1) * PD],
                                    w_tile[:, kc * D + ng * GW : kc * D + (ng + 1) * GW],
                                    start=(kc == 0),
                                    stop=(kc == NKC - 1),
                                )
                            sl = slice(ng * GW, (ng + 1) * GW)
                            if use_badd:
                                nc.vector.scalar_tensor_tensor(
                                    z[:, sl], pp[:], 1.0, res_tiles[tc_i][:, sl],
                                    op0=OP.mult, op1=OP.add,
                                )
                                nc.vector.scalar_tensor_tensor(
                                    z[:, sl], z[:, sl], 1.0, badd_t[:, sl],
                                    op0=OP.mult, op1=OP.add,
                                    accum_out=s01[:, ng : ng + 1],
                                )
                            else:
                                nc.vector.scalar_tensor_tensor(
                                    z[:, sl], pp[:], 1.0, res_tiles[tc_i][:, sl],
                                    op0=OP.mult, op1=OP.add,
                                    accum_out=s01[:, ng : ng + 1],
                                )
                        # layernorm over the full 768-wide row
                        ssum = p_sm.tile([PD, 1], F32, tag="ssum")
                        nc.vector.tensor_tensor(
                            ssum[:], s01[:, 0:1], s01[:, 1:2], op=OP.add
                        )
                        uneg = p_sm.tile([PD, 1], F32, tag="uneg")
                        nc.vector.tensor_scalar_mul(uneg[:], ssum[:], -1.0 / D)
                        sq = p_z.tile([PD, D], F32, tag="sq")
                        ssq = p_sm.tile([PD, 1], F32, tag="ssq")
                        nc.scalar.activation(
                            sq[:], z[:], AF.Square, bias=uneg[:], scale=1.0,
                            accum_out=ssq[:],
                        )
                        # rstd = exp(-0.5*ln(ssq/D)): Ln+Exp share one ACT
                        # table set with the attention Exp (Sqrt does not,
                        # and each set switch costs ~2.7us); /D folds into
                        # the Ln scale (reference eps=1e-12 is below fp32
                        # resolution for var ~ O(1), so it is dropped)
                        lnv = p_sm.tile([PD, 1], F32, tag="stdev")
                        nc.scalar.activation(
                            lnv[:], ssq[:], AF.Ln, bias=0.0, scale=1.0 / D
                        )
                        rstd = p_sm.tile([PD, 1], F32, tag="rstd")
                        nc.scalar.activation(
                            rstd[:], lnv[:], AF.Exp, bias=0.0, scale=-0.5
                        )
                        o = p_hid.tile([PD, D], F32R, tag=out_tag)
                        if use_ln:
                            on = p_z.tile([PD, D], F32, tag="sq")
                            nc.vector.tensor_scalar(
                                on[:], z[:], uneg[:], rstd[:], op0=OP.add, op1=OP.mult
                            )
                            nc.vector.tensor_tensor(
                                on[:], on[:], lnw_t[:], op=OP.mult
                            )
                            nc.vector.tensor_tensor(
                                o[:], on[:], lnb_t[:], op=OP.add
                            )
                        else:
                            nc.vector.tensor_scalar(
                                o[:], z[:], uneg[:], rstd[:], op0=OP.add, op1=OP.mult
                            )
                        if is_last:
                            nc.sync.dma_start(
                                out_d.ap()[tc_i * PD : (tc_i + 1) * PD, :], o[:]
                            )
                        outs.append(o)
                    return outs

                a_tiles = out_block(
                    ctxT, wo1_t, h_tiles, b1_d, use_b1,
                    ln1w_d, ln1b_d, use_ln1, "hid", False,
                )
                aT = transpose_norm_to_T(a_tiles, p_ht, "ht")
                h_tiles = out_block(
                    aT, wo2_t, a_tiles, b2_d, use_b2,
                    ln2w_d, ln2b_d, use_ln2, "hid", l == L - 1,
                )
                # last Q-proj chain lands after out2: it covers the PE
                # window where the next layer's hT transposes wait on the
                # final LN tiles
                qtn_chain()
                if l + 1 < L:
                    assert len(QT_next) == NKC
                    QT = QT_next

    if split_waits:
        import bass_rust

        _split_excess_waits(nc, mybir, bass_rust)
    return nc


def prep_inputs(inputs):
    """Host-side folds. Returns (flags, per-core-invariant map, per-core list)."""
    import ml_dtypes

    g = {k: np.asarray(v, dtype=np.float32) for k, v in inputs.items()}

    wq_s = g["Wq"] * SCALE
    bq_s = g["bq"] * SCALE
    b1 = np.einsum("ld,ldo->lo", g["bv"], g["Wo1"]) + g["bo1"]
    b2 = g["bo2"]

    flags = {
        "use_mask": bool(np.any(g["attention_mask"])),
        "use_bq": bool(np.any(bq_s)),
        "use_bk": bool(np.any(g["bk"])),
        "use_b1": bool(np.any(b1)),
        "use_b2": bool(np.any(b2)),
        "use_ln1": bool(np.any(g["ln1_w"] != 1.0) or np.any(g["ln1_b"])),
        "use_ln2": bool(np.any(g["ln2_w"] != 1.0) or np.any(g["ln2_b"])),
    }

    def wfmt(w):
        return np.ascontiguousarray(
            w.reshape(L, NKC, PD, D).transpose(0, 2, 1, 3).reshape(L, PD, NKC * D)
        ).astype(ml_dtypes.bfloat16)

    def bfmt(b):
        return np.ascontiguousarray(
            b.reshape(L, NKC, PD).transpose(2, 0, 1).reshape(PD, L * NKC)
        )

    shared = {
        "wq": wfmt(wq_s),
        "wk": wfmt(g["Wk"]),
        "wv": wfmt(g["Wv"]),
        "wo1": wfmt(g["Wo1"]),
        "wo2": wfmt(g["Wo2"]),
        "iden": np.eye(PD, dtype=np.float32),
    }
    if flags["use_bq"]:
        shared["bq"] = bfmt(bq_s)
    if flags["use_bk"]:
        shared["bk"] = bfmt(g["bk"])
    epair = np.zeros((H // 2, 3 * PD), dtype=ml_dtypes.bfloat16)
    for r in range(3):
        epair[2 * r, r * PD : r * PD + 64] = 1.0
        epair[2 * r + 1, r * PD + 64 : (r + 1) * PD] = 1.0
    shared["epair"] = epair
    shared["vones"] = np.ones((PD, H), dtype=np.float32)
    if flags["use_b1"]:
        shared["b1bc"] = np.ascontiguousarray(
            np.broadcast_to(b1[:, None, :], (L, PD, D))
        )
    if flags["use_b2"]:
        shared["b2bc"] = np.ascontiguousarray(
            np.broadcast_to(b2[:, None, :], (L, PD, D))
        )
    if flags["use_ln1"]:
        shared["ln1wbc"] = np.ascontiguousarray(
            np.broadcast_to(g["ln1_w"][:, None, :], (L, PD, D))
        )
        shared["ln1bbc"] = np.ascontiguousarray(
            np.broadcast_to(g["ln1_b"][:, None, :], (L, PD, D))
        )
    if flags["use_ln2"]:
        shared["ln2wbc"] = np.ascontiguousarray(
            np.broadcast_to(g["ln2_w"][:, None, :], (L, PD, D))
        )
        shared["ln2bbc"] = np.ascontiguousarray(
            np.broadcast_to(g["ln2_b"][:, None, :], (L, PD, D))
        )

    per_core = []
    for b in range(B):
        m = dict(shared)
        m["qs"] = np.ascontiguousarray(g["query_states"][b])
        m["hs"] = np.ascontiguousarray(g["hidden_states"][b])
        if flags["use_mask"]:
            m["mask"] = np.ascontiguousarray(
                g["attention_mask"][b].reshape(NTC, PD).T
            )
        per_core.append(m)
    return flags, per_core


TRACE = False
LAST_EXEC_NS = None
LAST_RESULTS = None


def kernel(**inputs):
    global LAST_EXEC_NS, LAST_RESULTS
    from concourse.bass_utils import run_bass_kernel_spmd

    flags, per_core = prep_inputs(inputs)
    nc = build_nc(flags)
    kw = {}
    if TRACE:
        kw = dict(trace=True, tmpdir="/root/problem/trace_out")
        import os

        os.makedirs("/root/problem/trace_out", exist_ok=True)
    res = run_bass_kernel_spmd(nc, per_core, core_ids=list(range(B)), **kw)
    LAST_EXEC_NS = res.exec_time_ns
    LAST_RESULTS = res
    out = np.stack([np.asarray(res.results[b]["out"]) for b in range(B)], axis=0)
    return out.astype(np.float32)

